# revision 6
# baseline (speedup 1.0000x reference)
"""GNN message-passing ConvNet layer on 8 TRN2 NeuronCores (Bass/Tile).

Computes, for x [B=4, N=4096, D=128], adj_mat [B, N, N] (0/1 floats),
U [D, D]:
    mask = (adj_mat > 0)
    deg[b, i] = sum_j adj_mat[b, j, i]
    agg[b, i, :] = sum_j mask[b, j, i] * x[b, j, :]
    out = relu((agg @ U) / deg[..., None])

Sharding: core c handles batch c//2 and destination-node half c%2 (the
column slice adj[b, :, i0:i0+2048]) — no collectives, identical per-core
work.

Per-core kernel, fp8 edition (the baseline moved adj as f32 and was
HBM-bound at ~350 GB/s):
  - adj is 0/1 so it is packed host-side to float8e4 (exact) — 8 MiB per
    core instead of 32 MiB. DRAM layout [128p][round][jtile][i] makes
    every DMA fully contiguous per partition.
  - x is split host-side into an fp8 hi/lo pair (x ~= hi + lo at ~bf16
    accuracy). Per 256-row j-pair two DoubleRow fp8 matmuls stream the
    same adj tile: stationary1 = x_hi (PSUM A), stationary2 =
    [ones | x_lo cols 1..127] (PSUM B). Column 0 of pass 2 makes
    B[0,:] = deg exactly, so degree costs no extra PE pass; dim 0 of
    x keeps hi-only precision (measured end-to-end rel err ~5e-3 vs the
    2e-2 gate).
  - DoubleRow processes 2 fp8 rows/cycle: 512 i-cols per matmul in ~512
    PE cycles, 32 matmuls per 512-i round.
  - Tail per round (emitted one round late so the PE FIFO never stalls):
    recip(B[0]) -> partition-broadcast -> (A+B)*rb on DVE -> one f32r
    U-matmul (stationary U, moving aggT, out [e,i]) -> ReLU -> bf16
    store [128e, 512i]; host transposes back.
"""

import os
import sys

for _p in ("/opt/trn_rl_repo",):
    if _p not in sys.path and os.path.isdir(_p):
        sys.path.insert(0, _p)

from contextlib import ExitStack

import numpy as np
import ml_dtypes

B, N, D = 4, 4096, 128
P = 128
N_CORES = 8
W = 512                 # destination columns per round (one PSUM bank)
I_CORE = N // 2         # destination columns per core
N_ROUNDS = I_CORE // W  # 4
NJT = N // P            # 32 j-tiles of 128 rows
NPAIR = NJT // 2        # 16 DoubleRow pairs of 256 rows

_PROG = None


def _build_program():
    from concourse import mybir, tile, bacc

    f32 = mybir.dt.float32
    f32r = mybir.dt.float32r
    bf16 = mybir.dt.bfloat16
    fp8 = mybir.dt.float8e4
    DR = mybir.MatmulPerfMode.DoubleRow

    nc = bacc.Bacc(
        "TRN2",
        target_bir_lowering=False,
        debug=False,
        enable_asserts=True,
        num_devices=N_CORES,
    )
    # [p][round][jtile][i] — per partition each round's block is 16 KiB
    # contiguous, so every chunk DMA is clean per-partition runs.
    adj_d = nc.dram_tensor("adj_p", [P, N_ROUNDS, NJT, W], fp8, kind="ExternalInput")
    # hi/lo stationaries packed together: [p][2][jtile][d]
    xs_d = nc.dram_tensor("xs_p", [P, 2, NJT, D], fp8, kind="ExternalInput")
    u_d = nc.dram_tensor("U", [D, D], f32r, kind="ExternalInput")
    # output [e, i_core] bf16; host transposes/upcasts.
    out_d = nc.dram_tensor("out_t", [P, I_CORE], bf16, kind="ExternalOutput")

    with tile.TileContext(nc, trace_sim=False) as tc, ExitStack() as ctx:
        const_pool = ctx.enter_context(tc.tile_pool(name="const", bufs=1))
        adj_pool = ctx.enter_context(tc.tile_pool(name="adj", bufs=6))
        scale_pool = ctx.enter_context(tc.tile_pool(name="scale", bufs=2))
        out_pool = ctx.enter_context(tc.tile_pool(name="out", bufs=2))
        small_pool = ctx.enter_context(tc.tile_pool(name="small", bufs=2))
        ps_a = ctx.enter_context(tc.tile_pool(name="ps_a", bufs=3, space="PSUM"))
        ps_b = ctx.enter_context(tc.tile_pool(name="ps_b", bufs=3, space="PSUM"))
        ps_o = ctx.enter_context(tc.tile_pool(name="ps_o", bufs=2, space="PSUM"))

        xs_sb = const_pool.tile([P, 2, NJT, D], fp8)
        nc.scalar.dma_start(xs_sb[:], xs_d[:])
        u_sb = const_pool.tile([P, D], f32r)
        nc.scalar.dma_start(u_sb[:], u_d[:])

        def emit_tail(q, a_ps, b_ps):
            """Round tail: combine hi+lo, 1/deg scale, U-matmul, ReLU, store."""
            recip = small_pool.tile([1, W], f32, tag="recip")
            nc.vector.reciprocal_approx_fast(recip[:], b_ps[0:1, :])
            rb = scale_pool.tile([P, W], f32, tag="rb")
            nc.gpsimd.partition_broadcast(rb[:], recip[:])
            # sum = A + B everywhere, then re-copy row 0 from A alone
            # (B[0,:] is deg, not a lo-correction).
            sum_sb = scale_pool.tile([P, W], f32, tag="sum")
            nc.vector.tensor_copy(sum_sb[:], a_ps[:])
            nc.vector.tensor_add(sum_sb[:], sum_sb[:], b_ps[:])
            nc.vector.tensor_copy(sum_sb[0:1, :], a_ps[0:1, :])
            aggs = scale_pool.tile([P, W], f32r, tag="aggs")
            nc.vector.tensor_mul(aggs[:], sum_sb[:], rb[:])
            o_ps = ps_o.tile([P, W], f32, tag="ops")
            nc.tensor.matmul(o_ps[:], u_sb[:], aggs[:], start=True, stop=True)
            out_sb = out_pool.tile([P, W], bf16, tag="osb")
            nc.vector.tensor_relu(out_sb[:], o_ps[:])
            nc.scalar.dma_start(out_d[:, q * W : (q + 1) * W], out_sb[:])

        pending = None
        for q in range(N_ROUNDS):
            a_ps = ps_a.tile([P, W], f32, tag="a")
            b_ps = ps_b.tile([P, W], f32, tag="b")
            # Last round streams in smaller chunks so the compute trailing
            # the final DMA is as short as possible.
            if q == N_ROUNDS - 1:
                chunk_pairs = [2] * (NPAIR // 2)
            else:
                chunk_pairs = [4] * (NPAIR // 4)
            pt0 = 0
            for c, cp in enumerate(chunk_pairs):
                adj_sb = adj_pool.tile([P, 2 * cp, W], fp8, tag="adj")
                nc.sync.dma_start(
                    adj_sb[:], adj_d[:, q, 2 * pt0 : 2 * (pt0 + cp), :]
                )
                first, last = c == 0, c == len(chunk_pairs) - 1
                for u in range(cp):
                    pt = pt0 + u
                    mv = adj_sb[:, 2 * u : 2 * u + 2, :]
                    nc.tensor.matmul(
                        a_ps[:],
                        xs_sb[:, 0, 2 * pt : 2 * pt + 2, :],
                        mv,
                        start=(first and u == 0),
                        stop=(last and u == cp - 1),
                        perf_mode=DR,
                    )
                    nc.tensor.matmul(
                        b_ps[:],
                        xs_sb[:, 1, 2 * pt : 2 * pt + 2, :],
                        mv,
                        start=(first and u == 0),
                        stop=(last and u == cp - 1),
                        perf_mode=DR,
                    )
                pt0 += cp
            if pending is not None:
                emit_tail(*pending)
            pending = (q, a_ps, b_ps)
        emit_tail(*pending)

    nc.compile()
    return nc


def _get_program():
    global _PROG
    if _PROG is None:
        _PROG = _build_program()
    return _PROG


E4 = ml_dtypes.float8_e4m3


def _shard_inputs(x, adj_mat, U):
    # adj -> fp8 via bit trick: 0/1 exact (1.0 == 0x38 in e4m3).
    adj8 = (adj_mat != 0).astype(np.uint8) * np.uint8(0x38)
    x32 = np.asarray(x, dtype=np.float32)
    in_maps = []
    for c in range(N_CORES):
        b, half = c // 2, c % 2
        i0 = half * I_CORE
        a = adj8[b, :, i0 : i0 + I_CORE]  # [N, I_CORE] uint8
        # [t*128+p, q*512+i] -> [p, q, t, i]
        a = np.ascontiguousarray(
            a.reshape(NJT, P, N_ROUNDS, W).transpose(1, 2, 0, 3)
        ).view(E4)
        xb = x32[b]  # [N, D]
        xh = xb.astype(E4)
        xl = (xb - xh.astype(np.float32)).astype(E4)
        xl[:, 0] = E4(1.0)  # ones column -> deg in PSUM B partition 0
        xs = np.stack([xh, xl], axis=0)  # [2, N, D]
        # [s, t*128+p, d] -> [p, s, t, d]
        xs = np.ascontiguousarray(
            xs.reshape(2, NJT, P, D).transpose(2, 0, 1, 3)
        )
        in_maps.append({"adj_p": a, "xs_p": xs, "U": np.ascontiguousarray(U)})
    return in_maps


def _run(x, adj_mat, U, trace=False):
    from concourse.bass_utils import run_bass_kernel_spmd

    nc = _get_program()
    in_maps = _shard_inputs(x, adj_mat, U)
    res = run_bass_kernel_spmd(
        nc, in_maps, core_ids=list(range(N_CORES)), trace=trace
    )
    out = np.empty((B, N, D), dtype=np.float32)
    for c in range(N_CORES):
        b, half = c // 2, c % 2
        i0 = half * I_CORE
        ot = res.results[c]["out_t"]  # [128 e, I_CORE] bf16
        out[b, i0 : i0 + I_CORE, :] = ot.astype(np.float32).T
    return out, res


def kernel(x, adj_mat, U):
    out, _ = _run(
        np.asarray(x, dtype=np.float32),
        np.asarray(adj_mat, dtype=np.float32),
        np.asarray(U, dtype=np.float32),
    )
    return out


# revision 13
# speedup vs baseline: 1.0133x; 1.0133x over previous
"""GNN message-passing ConvNet layer on 8 TRN2 NeuronCores (Bass/Tile).

Computes, for x [B=4, N=4096, D=128], adj_mat [B, N, N] (0/1 floats),
U [D, D]:
    mask = (adj_mat > 0)
    deg[b, i] = sum_j adj_mat[b, j, i]
    agg[b, i, :] = sum_j mask[b, j, i] * x[b, j, :]
    out = relu((agg @ U) / deg[..., None])

Sharding: core c handles batch c//2 and destination-node half c%2 (the
column slice adj[b, :, i0:i0+2048]) — no collectives, identical per-core
work.

Per-core kernel, fp8 edition (the f32 baseline was HBM-bound at
~350 GB/s):
  - adj is 0/1 so it is packed host-side to float8e4 (exact) — 8 MiB per
    core instead of 32 MiB. DRAM layout [128p][round][jtile][i] makes
    every DMA fully contiguous per partition.
  - x is split host-side into an fp8 hi/lo pair (x ~= hi + lo at ~bf16
    accuracy). Per 256-row j-pair two DoubleRow fp8 matmuls stream the
    same adj tile: stationary1 = x_hi (PSUM A), stationary2 =
    [ones | x_lo cols 1..127] (PSUM B). Column 0 of pass 2 makes
    B[0,:] = deg exactly, so degree costs no extra PE pass; dim 0 of
    x keeps hi-only precision (measured end-to-end rel err ~5e-3 vs the
    2e-2 gate).
  - DoubleRow contracts 256 rows/output-column; the stream advances at
    1 output column/cycle, so the whole agg+deg stream is ~65k PE cycles.
  - Startup: x(hi/lo) head + first small adj chunks lead the sync queue
    so real matmuls start as soon as possible; 8 warmup matmuls on a
    zeroed scratch tile flip the PE HAM clock-gate to 2.4 GHz before the
    real stream arrives.
  - Tail per round: recip(B[0]) -> partition-broadcast -> fused
    (A+B) via scalar_tensor_tensor -> *recip -> one f32r U-matmul
    (stationary U, moving aggT, out [e,i]) -> ReLU -> bf16 store.
    Tails are emitted two chunks into the next round (late enough that
    the PE FIFO never stalls, early enough that they don't pile up at
    the end); the last round's tail is split into two column halves so
    its serial chain pipelines.
"""

import os
import sys

for _p in ("/opt/trn_rl_repo",):
    if _p not in sys.path and os.path.isdir(_p):
        sys.path.insert(0, _p)

from contextlib import ExitStack

import numpy as np
import ml_dtypes

B, N, D = 4, 4096, 128
P = 128
N_CORES = 8
W = 512                 # destination columns per round (one PSUM bank)
I_CORE = N // 2         # destination columns per core
N_ROUNDS = I_CORE // W  # 4
NJT = N // P            # 32 j-tiles of 128 rows
NPAIR = NJT // 2        # 16 DoubleRow pairs of 256 rows
XS_HEAD = 4             # pairs of x loaded before the first adj chunk
N_WARM = 8              # warmup matmuls to heat the PE clock-gate

_PROG = None


def _build_program():
    from concourse import mybir, tile, bacc

    f32 = mybir.dt.float32
    f32r = mybir.dt.float32r
    bf16 = mybir.dt.bfloat16
    fp8 = mybir.dt.float8e4
    DR = mybir.MatmulPerfMode.DoubleRow
    MUL = mybir.AluOpType.mult
    ADD = mybir.AluOpType.add
    RELU = mybir.ActivationFunctionType.Relu

    nc = bacc.Bacc(
        "TRN2",
        target_bir_lowering=False,
        debug=False,
        enable_asserts=True,
        num_devices=N_CORES,
    )
    # [p][round][jtile][i] — per partition each round's block is 16 KiB
    # contiguous, so every chunk DMA is clean per-partition runs.
    adj_d = nc.dram_tensor("adj_p", [P, N_ROUNDS, NJT, W], fp8, kind="ExternalInput")
    # hi/lo stationaries interleaved by j-tile: [p][jtile][2][d] so a
    # leading slice of j-tiles is contiguous (head/tail split load).
    xs_d = nc.dram_tensor("xs_p", [P, NJT, 2, D], fp8, kind="ExternalInput")
    u_d = nc.dram_tensor("U", [D, D], f32r, kind="ExternalInput")
    # -U[0, :] as a per-partition bias column: the deg row rides through
    # the combine as "+1", the U-matmul turns it into +U[0,e], and the
    # ReLU's bias subtracts it back out.
    u0n_d = nc.dram_tensor("u0n", [D, 1], f32, kind="ExternalInput")
    # output [e, i_core] bf16; host transposes/upcasts.
    out_d = nc.dram_tensor("out_t", [P, I_CORE], bf16, kind="ExternalOutput")

    with tile.TileContext(nc, trace_sim=False) as tc, ExitStack() as ctx:
        const_pool = ctx.enter_context(tc.tile_pool(name="const", bufs=1))
        adj_pool = ctx.enter_context(tc.tile_pool(name="adj", bufs=6))
        scale_pool = ctx.enter_context(tc.tile_pool(name="scale", bufs=2))
        out_pool = ctx.enter_context(tc.tile_pool(name="out", bufs=2))
        small_pool = ctx.enter_context(tc.tile_pool(name="small", bufs=2))
        ps_a = ctx.enter_context(tc.tile_pool(name="ps_a", bufs=3, space="PSUM"))
        ps_b = ctx.enter_context(tc.tile_pool(name="ps_b", bufs=3, space="PSUM"))
        ps_o = ctx.enter_context(tc.tile_pool(name="ps_o", bufs=1, space="PSUM"))
        ps_w = ctx.enter_context(tc.tile_pool(name="ps_w", bufs=1, space="PSUM"))

        # --- warmup: flip the PE HAM clock-gate while DMAs stage ---
        warm_sb = const_pool.tile([P, 2, W], fp8)
        nc.vector.memset(warm_sb[:], 0.0)
        warm_ps = ps_w.tile([P, W], f32, tag="warm")
        for _ in range(N_WARM):
            nc.tensor.matmul(
                warm_ps[:],
                warm_sb[:, :, 0:D],
                warm_sb[:],
                start=True,
                stop=True,
                perf_mode=DR,
            )

        # --- constant loads: x head first (gates the first matmuls) ---
        xs_sb = const_pool.tile([P, NJT, 2, D], fp8)
        nc.sync.dma_start(xs_sb[:, 0 : 2 * XS_HEAD], xs_d[:, 0 : 2 * XS_HEAD])
        u_sb = const_pool.tile([P, D], f32r)
        nc.scalar.dma_start(u_sb[:], u_d[:])
        u0n_sb = const_pool.tile([P, 1], f32)
        nc.scalar.dma_start(u0n_sb[:], u0n_d[:])

        def emit_tail(q, a_ps, b_ps, split):
            """Round tail: combine hi+lo, 1/deg scale, U-matmul, ReLU, store.
            split=2 pipelines the chain in two column halves."""
            ws = W // split
            o_ps = ps_o.tile([P, W], f32, tag="ops")
            for h in range(split):
                cs = slice(h * ws, (h + 1) * ws)
                # A lands one matmul before B; its PSUM->SBUF copy can
                # start while B's last matmul drains.
                acp = scale_pool.tile([P, ws], f32, tag=f"acp{split}{h}")
                nc.vector.tensor_copy(acp[:], a_ps[:, cs])
                recip = small_pool.tile([1, ws], f32, tag=f"recip{split}{h}")
                nc.vector.reciprocal_approx_fast(recip[:], b_ps[0:1, cs])
                rb = scale_pool.tile([P, ws], f32, tag=f"rb{split}{h}")
                nc.gpsimd.partition_broadcast(rb[:], recip[:])
                # sum = A + B; row 0 becomes A[0] + deg, handled by the
                # ReLU bias below.
                sum_sb = scale_pool.tile([P, ws], f32, tag=f"sum{split}{h}")
                nc.vector.scalar_tensor_tensor(
                    sum_sb[:], b_ps[:, cs], 1.0, acp[:], MUL, ADD
                )
                aggs = scale_pool.tile([P, ws], f32r, tag=f"aggs{split}{h}")
                nc.vector.tensor_mul(aggs[:], sum_sb[:], rb[:])
                nc.tensor.matmul(o_ps[:, cs], u_sb[:], aggs[:], start=True, stop=True)
                out_sb = out_pool.tile([P, ws], bf16, tag=f"osb{split}{h}")
                nc.scalar.activation(
                    out_sb[:], o_ps[:, cs], RELU, bias=u0n_sb[:]
                )
                nc.scalar.dma_start(out_d[:, q * W + h * ws : q * W + (h + 1) * ws], out_sb[:])

        CHUNKS = [
            [2, 2, 4, 8],
            [8, 8],
            [8, 8],
            [8, 4, 2, 2],
        ]
        pending = None
        for q in range(N_ROUNDS):
            a_ps = ps_a.tile([P, W], f32, tag="a")
            b_ps = ps_b.tile([P, W], f32, tag="b")
            chunk_pairs = CHUNKS[q]
            pt0 = 0
            for c, cp in enumerate(chunk_pairs):
                adj_sb = adj_pool.tile([P, 2 * cp, W], fp8, tag="adj")
                nc.sync.dma_start(
                    adj_sb[:], adj_d[:, q, 2 * pt0 : 2 * (pt0 + cp), :]
                )
                if q == 0 and c == 0:
                    # rest of x rides the same queue right after chunk 0
                    nc.sync.dma_start(
                        xs_sb[:, 2 * XS_HEAD :], xs_d[:, 2 * XS_HEAD :]
                    )
                first, last = c == 0, c == len(chunk_pairs) - 1
                for u in range(cp):
                    pt = pt0 + u
                    mv = adj_sb[:, 2 * u : 2 * u + 2, :]
                    nc.tensor.matmul(
                        a_ps[:],
                        xs_sb[:, 2 * pt : 2 * pt + 2, 0, :],
                        mv,
                        start=(first and u == 0),
                        stop=(last and u == cp - 1),
                        perf_mode=DR,
                    )
                    nc.tensor.matmul(
                        b_ps[:],
                        xs_sb[:, 2 * pt : 2 * pt + 2, 1, :],
                        mv,
                        start=(first and u == 0),
                        stop=(last and u == cp - 1),
                        perf_mode=DR,
                    )
                pt0 += cp
                if pending is not None and c == 1:
                    emit_tail(*pending, split=1)
                    pending = None
            pending = (q, a_ps, b_ps)
        emit_tail(*pending, split=2)

    nc.compile()
    return nc


def _get_program():
    global _PROG
    if _PROG is None:
        _PROG = _build_program()
    return _PROG


E4 = ml_dtypes.float8_e4m3


def _shard_inputs(x, adj_mat, U):
    # adj -> fp8 via bit trick: 0/1 exact (1.0 == 0x38 in e4m3).
    adj8 = (adj_mat != 0).astype(np.uint8) * np.uint8(0x38)
    x32 = np.asarray(x, dtype=np.float32)
    in_maps = []
    for c in range(N_CORES):
        b, half = c // 2, c % 2
        i0 = half * I_CORE
        a = adj8[b, :, i0 : i0 + I_CORE]  # [N, I_CORE] uint8
        # [t*128+p, q*512+i] -> [p, q, t, i]
        a = np.ascontiguousarray(
            a.reshape(NJT, P, N_ROUNDS, W).transpose(1, 2, 0, 3)
        ).view(E4)
        xb = x32[b]  # [N, D]
        xh = xb.astype(E4)
        xl = (xb - xh.astype(np.float32)).astype(E4)
        xl[:, 0] = E4(1.0)  # ones column -> deg in PSUM B partition 0
        xs = np.stack([xh, xl], axis=1)  # [N, 2, D]
        # [t*128+p, s, d] -> [p, t, s, d]
        xs = np.ascontiguousarray(
            xs.reshape(NJT, P, 2, D).transpose(1, 0, 2, 3)
        )
        u32 = np.ascontiguousarray(U.astype(np.float32))
        u0n = np.ascontiguousarray(-u32[0, :].reshape(D, 1))
        in_maps.append({"adj_p": a, "xs_p": xs, "U": u32, "u0n": u0n})
    return in_maps


def _run(x, adj_mat, U, trace=False):
    from concourse.bass_utils import run_bass_kernel_spmd

    nc = _get_program()
    in_maps = _shard_inputs(x, adj_mat, U)
    res = run_bass_kernel_spmd(
        nc, in_maps, core_ids=list(range(N_CORES)), trace=trace
    )
    out = np.empty((B, N, D), dtype=np.float32)
    for c in range(N_CORES):
        b, half = c // 2, c % 2
        i0 = half * I_CORE
        ot = res.results[c]["out_t"]  # [128 e, I_CORE] bf16
        out[b, i0 : i0 + I_CORE, :] = ot.astype(np.float32).T
    return out, res


def kernel(x, adj_mat, U):
    out, _ = _run(
        np.asarray(x, dtype=np.float32),
        np.asarray(adj_mat, dtype=np.float32),
        np.asarray(U, dtype=np.float32),
    )
    return out


# revision 17
# speedup vs baseline: 1.0363x; 1.0227x over previous
"""GNN message-passing ConvNet layer on 8 TRN2 NeuronCores (Bass/Tile).

Computes, for x [B=4, N=4096, D=128], adj_mat [B, N, N] (0/1 floats),
U [D, D]:
    mask = (adj_mat > 0)
    deg[b, i] = sum_j adj_mat[b, j, i]
    agg[b, i, :] = sum_j mask[b, j, i] * x[b, j, :]
    out = relu((agg @ U) / deg[..., None])

Sharding: core c handles batch c//2 and destination-node half c%2 (the
column slice adj[b, :, i0:i0+2048]) — no collectives, identical per-core
work.

Per-core kernel, fp8 edition (the f32 baseline was HBM-bound at
~350 GB/s):
  - adj is 0/1 so it is packed host-side to float8e4 (exact) — 8 MiB per
    core instead of 32 MiB. DRAM layout [128p][round][jtile][i] makes
    every DMA fully contiguous per partition.
  - x is split host-side into an fp8 hi/lo pair (x ~= hi + lo at ~bf16
    accuracy). Per 256-row j-pair two DoubleRow fp8 matmuls stream the
    same adj tile: stationary1 = x_hi (PSUM A), stationary2 =
    [ones | x_lo cols 1..127] (PSUM B). Column 0 of pass 2 makes
    B[0,:] = deg exactly, so degree costs no extra PE pass; dim 0 of
    x keeps hi-only precision (measured end-to-end rel err ~5e-3 vs the
    2e-2 gate).
  - DoubleRow contracts 256 rows/output-column; the stream advances at
    1 output column/cycle, so the whole agg+deg stream is ~65k PE cycles.
  - Startup: x(hi/lo) head + first small adj chunks lead the sync queue
    so real matmuls start as soon as possible; 8 warmup matmuls on a
    zeroed scratch tile flip the PE HAM clock-gate to 2.4 GHz before the
    real stream arrives.
  - Tail per round: recip(B[0]) -> partition-broadcast -> fused
    (A+B) via scalar_tensor_tensor -> *recip -> one f32r U-matmul
    (stationary U, moving aggT, out [e,i]) -> ReLU -> bf16 store.
    Tails are emitted two chunks into the next round (late enough that
    the PE FIFO never stalls, early enough that they don't pile up at
    the end); the last round's tail is split into two column halves so
    its serial chain pipelines.
"""

import os
import sys

for _p in ("/opt/trn_rl_repo",):
    if _p not in sys.path and os.path.isdir(_p):
        sys.path.insert(0, _p)

from contextlib import ExitStack

import numpy as np
import ml_dtypes

B, N, D = 4, 4096, 128
P = 128
N_CORES = 8
W = 512                 # destination columns per round (one PSUM bank)
I_CORE = N // 2         # destination columns per core
N_ROUNDS = I_CORE // W  # 4
NJT = N // P            # 32 j-tiles of 128 rows
NPAIR = NJT // 2        # 16 DoubleRow pairs of 256 rows
XS_HEAD = 4             # pairs of x loaded before the first adj chunk

_PROG = None


def _build_program():
    from concourse import mybir, tile, bacc

    f32 = mybir.dt.float32
    f32r = mybir.dt.float32r
    bf16 = mybir.dt.bfloat16
    fp8 = mybir.dt.float8e4
    DR = mybir.MatmulPerfMode.DoubleRow
    MUL = mybir.AluOpType.mult
    ADD = mybir.AluOpType.add
    RELU = mybir.ActivationFunctionType.Relu

    nc = bacc.Bacc(
        "TRN2",
        target_bir_lowering=False,
        debug=False,
        enable_asserts=False,
        num_devices=N_CORES,
    )
    # [p][round][jtile][i] — per partition each round's block is 16 KiB
    # contiguous, so every chunk DMA is clean per-partition runs.
    adj_d = nc.dram_tensor("adj_p", [P, N_ROUNDS, NJT, W], fp8, kind="ExternalInput")
    # hi/lo stationaries interleaved by j-tile: [p][jtile][2][d] so a
    # leading slice of j-tiles is contiguous (head/tail split load).
    xs_d = nc.dram_tensor("xs_p", [P, NJT, 2, D], fp8, kind="ExternalInput")
    u_d = nc.dram_tensor("U", [D, D], f32r, kind="ExternalInput")
    # -U[0, :] as a per-partition bias column: the deg row rides through
    # the combine as "+1", the U-matmul turns it into +U[0,e], and the
    # ReLU's bias subtracts it back out.
    u0n_d = nc.dram_tensor("u0n", [D, 1], f32, kind="ExternalInput")
    # output [e, i_core] bf16; host transposes/upcasts.
    out_d = nc.dram_tensor("out_t", [P, I_CORE], bf16, kind="ExternalOutput")

    with tile.TileContext(nc, trace_sim=False) as tc, ExitStack() as ctx:
        const_pool = ctx.enter_context(tc.tile_pool(name="const", bufs=1))
        adj_pool = ctx.enter_context(tc.tile_pool(name="adj", bufs=8))
        scale_pool = ctx.enter_context(tc.tile_pool(name="scale", bufs=2))
        out_pool = ctx.enter_context(tc.tile_pool(name="out", bufs=2))
        small_pool = ctx.enter_context(tc.tile_pool(name="small", bufs=2))
        ps_a = ctx.enter_context(tc.tile_pool(name="ps_a", bufs=3, space="PSUM"))
        ps_b = ctx.enter_context(tc.tile_pool(name="ps_b", bufs=3, space="PSUM"))
        ps_o = ctx.enter_context(tc.tile_pool(name="ps_o", bufs=1, space="PSUM"))

        # --- constant loads: x head leads the adj (sync) queue so the
        # first matmuls are gated only by it + chunk 0; the x tail and U
        # ride the scalar queue in parallel. ---
        xs_sb = const_pool.tile([P, NJT, 2, D], fp8)
        nc.sync.dma_start(xs_sb[:, 0 : 2 * XS_HEAD], xs_d[:, 0 : 2 * XS_HEAD])
        u_sb = const_pool.tile([P, D], f32r)
        nc.scalar.dma_start(u_sb[:], u_d[:])
        u0n_sb = const_pool.tile([P, 1], f32)
        nc.scalar.dma_start(u0n_sb[:], u0n_d[:])
        nc.scalar.dma_start(
            xs_sb[:, 2 * XS_HEAD : 16], xs_d[:, 2 * XS_HEAD : 16]
        )
        nc.scalar.dma_start(xs_sb[:, 16:], xs_d[:, 16:])

        def emit_tail(q, a_ps, b_ps, split):
            """Round tail: combine hi+lo, 1/deg scale, U-matmul, ReLU, store.
            split=2 pipelines the chain in two column halves."""
            ws = W // split
            o_ps = ps_o.tile([P, W], f32, tag="ops")
            for h in range(split):
                cs = slice(h * ws, (h + 1) * ws)
                # A lands one matmul before B; its PSUM->SBUF copy can
                # start while B's last matmul drains.
                acp = scale_pool.tile([P, ws], f32, tag=f"acp{split}{h}")
                nc.vector.tensor_copy(acp[:], a_ps[:, cs])
                recip = small_pool.tile([1, ws], f32, tag=f"recip{split}{h}")
                nc.vector.reciprocal_approx_fast(recip[:], b_ps[0:1, cs])
                rb = scale_pool.tile([P, ws], f32, tag=f"rb{split}{h}")
                nc.gpsimd.partition_broadcast(rb[:], recip[:])
                # sum = A + B; row 0 becomes A[0] + deg, handled by the
                # ReLU bias below.
                sum_sb = scale_pool.tile([P, ws], f32, tag=f"sum{split}{h}")
                nc.vector.scalar_tensor_tensor(
                    sum_sb[:], b_ps[:, cs], 1.0, acp[:], MUL, ADD
                )
                aggs = scale_pool.tile([P, ws], f32r, tag=f"aggs{split}{h}")
                nc.vector.tensor_mul(aggs[:], sum_sb[:], rb[:])
                nc.tensor.matmul(o_ps[:, cs], u_sb[:], aggs[:], start=True, stop=True)
                out_sb = out_pool.tile([P, ws], bf16, tag=f"osb{split}{h}")
                nc.scalar.activation(
                    out_sb[:], o_ps[:, cs], RELU, bias=u0n_sb[:]
                )
                nc.scalar.dma_start(out_d[:, q * W + h * ws : q * W + (h + 1) * ws], out_sb[:])

        CHUNKS = [
            [2, 2, 4, 8],
            [8, 8],
            [8, 8],
            [8, 4, 2, 2],
        ]
        pending = None
        for q in range(N_ROUNDS):
            a_ps = ps_a.tile([P, W], f32, tag="a")
            b_ps = ps_b.tile([P, W], f32, tag="b")
            chunk_pairs = CHUNKS[q]
            pt0 = 0
            for c, cp in enumerate(chunk_pairs):
                adj_sb = adj_pool.tile([P, 2 * cp, W], fp8, tag="adj")
                nc.sync.dma_start(
                    adj_sb[:], adj_d[:, q, 2 * pt0 : 2 * (pt0 + cp), :]
                )
                first, last = c == 0, c == len(chunk_pairs) - 1
                for u in range(cp):
                    pt = pt0 + u
                    mv = adj_sb[:, 2 * u : 2 * u + 2, :]
                    nc.tensor.matmul(
                        a_ps[:],
                        xs_sb[:, 2 * pt : 2 * pt + 2, 0, :],
                        mv,
                        start=(first and u == 0),
                        stop=(last and u == cp - 1),
                        perf_mode=DR,
                    )
                    nc.tensor.matmul(
                        b_ps[:],
                        xs_sb[:, 2 * pt : 2 * pt + 2, 1, :],
                        mv,
                        start=(first and u == 0),
                        stop=(last and u == cp - 1),
                        perf_mode=DR,
                    )
                pt0 += cp
                if pending is not None and c == 1:
                    emit_tail(*pending, split=1)
                    pending = None
            pending = (q, a_ps, b_ps)
        emit_tail(*pending, split=2)

    nc.compile()
    return nc


def _get_program():
    global _PROG
    if _PROG is None:
        _PROG = _build_program()
    return _PROG


E4 = ml_dtypes.float8_e4m3


def _shard_inputs(x, adj_mat, U):
    # adj -> fp8 via bit trick: 0/1 exact (1.0 == 0x38 in e4m3).
    adj8 = (adj_mat != 0).astype(np.uint8) * np.uint8(0x38)
    x32 = np.asarray(x, dtype=np.float32)
    in_maps = []
    for c in range(N_CORES):
        b, half = c // 2, c % 2
        i0 = half * I_CORE
        a = adj8[b, :, i0 : i0 + I_CORE]  # [N, I_CORE] uint8
        # [t*128+p, q*512+i] -> [p, q, t, i]
        a = np.ascontiguousarray(
            a.reshape(NJT, P, N_ROUNDS, W).transpose(1, 2, 0, 3)
        ).view(E4)
        xb = x32[b]  # [N, D]
        xh = xb.astype(E4)
        xl = (xb - xh.astype(np.float32)).astype(E4)
        xl[:, 0] = E4(1.0)  # ones column -> deg in PSUM B partition 0
        xs = np.stack([xh, xl], axis=1)  # [N, 2, D]
        # [t*128+p, s, d] -> [p, t, s, d]
        xs = np.ascontiguousarray(
            xs.reshape(NJT, P, 2, D).transpose(1, 0, 2, 3)
        )
        u32 = np.ascontiguousarray(U.astype(np.float32))
        u0n = np.ascontiguousarray(-u32[0, :].reshape(D, 1))
        in_maps.append({"adj_p": a, "xs_p": xs, "U": u32, "u0n": u0n})
    return in_maps


def _run(x, adj_mat, U, trace=False):
    from concourse.bass_utils import run_bass_kernel_spmd

    nc = _get_program()
    in_maps = _shard_inputs(x, adj_mat, U)
    res = run_bass_kernel_spmd(
        nc, in_maps, core_ids=list(range(N_CORES)), trace=trace
    )
    out = np.empty((B, N, D), dtype=np.float32)
    for c in range(N_CORES):
        b, half = c // 2, c % 2
        i0 = half * I_CORE
        ot = res.results[c]["out_t"]  # [128 e, I_CORE] bf16
        out[b, i0 : i0 + I_CORE, :] = ot.astype(np.float32).T
    return out, res


def kernel(x, adj_mat, U):
    out, _ = _run(
        np.asarray(x, dtype=np.float32),
        np.asarray(adj_mat, dtype=np.float32),
        np.asarray(U, dtype=np.float32),
    )
    return out


# revision 19
# speedup vs baseline: 1.0424x; 1.0059x over previous
"""GNN message-passing ConvNet layer on 8 TRN2 NeuronCores (Bass/Tile).

Computes, for x [B=4, N=4096, D=128], adj_mat [B, N, N] (0/1 floats),
U [D, D]:
    mask = (adj_mat > 0)
    deg[b, i] = sum_j adj_mat[b, j, i]
    agg[b, i, :] = sum_j mask[b, j, i] * x[b, j, :]
    out = relu((agg @ U) / deg[..., None])

Sharding: core c handles batch c//2 and destination-node half c%2 (the
column slice adj[b, :, i0:i0+2048]) — no collectives, identical per-core
work.

Per-core kernel, fp8 edition (the f32 baseline was HBM-bound at
~350 GB/s):
  - adj is 0/1 so it is packed host-side to float8e4 (exact) — 8 MiB per
    core instead of 32 MiB. DRAM layout [128p][round][jtile][i] makes
    every DMA fully contiguous per partition.
  - x is split host-side into an fp8 hi/lo pair (x ~= hi + lo at ~bf16
    accuracy). Per 256-row j-pair two DoubleRow fp8 matmuls stream the
    same adj tile: stationary1 = x_hi (PSUM A), stationary2 =
    [ones | x_lo cols 1..127] (PSUM B). Column 0 of pass 2 makes
    B[0,:] = deg exactly, so degree costs no extra PE pass; dim 0 of
    x keeps hi-only precision (measured end-to-end rel err ~5e-3 vs the
    2e-2 gate).
  - DoubleRow contracts 256 rows/output-column; the stream advances at
    1 output column/cycle, so the whole agg+deg stream is ~65k PE cycles.
  - Startup: x(hi/lo) head + first small adj chunks lead the sync queue
    so real matmuls start as soon as possible; 8 warmup matmuls on a
    zeroed scratch tile flip the PE HAM clock-gate to 2.4 GHz before the
    real stream arrives.
  - Tail per round: recip(B[0]) -> partition-broadcast -> fused
    (A+B) via scalar_tensor_tensor -> *recip -> one f32r U-matmul
    (stationary U, moving aggT, out [e,i]) -> ReLU -> bf16 store.
    Tails are emitted two chunks into the next round (late enough that
    the PE FIFO never stalls, early enough that they don't pile up at
    the end); the last round's tail is split into two column halves so
    its serial chain pipelines.
"""

import os
import sys

for _p in ("/opt/trn_rl_repo",):
    if _p not in sys.path and os.path.isdir(_p):
        sys.path.insert(0, _p)

from contextlib import ExitStack

import numpy as np
import ml_dtypes

B, N, D = 4, 4096, 128
P = 128
N_CORES = 8
W = 512                 # destination columns per round (one PSUM bank)
I_CORE = N // 2         # destination columns per core
N_ROUNDS = I_CORE // W  # 4
NJT = N // P            # 32 j-tiles of 128 rows
NPAIR = NJT // 2        # 16 DoubleRow pairs of 256 rows
XS_HEAD = 4             # pairs of x loaded before the first adj chunk

_PROG = None


def _build_program():
    from concourse import mybir, tile, bacc

    f32 = mybir.dt.float32
    f32r = mybir.dt.float32r
    bf16 = mybir.dt.bfloat16
    fp8 = mybir.dt.float8e4
    DR = mybir.MatmulPerfMode.DoubleRow
    MUL = mybir.AluOpType.mult
    ADD = mybir.AluOpType.add
    RELU = mybir.ActivationFunctionType.Relu

    nc = bacc.Bacc(
        "TRN2",
        target_bir_lowering=False,
        debug=False,
        enable_asserts=False,
        num_devices=N_CORES,
    )
    # [p][round][jtile][i] — per partition each round's block is 16 KiB
    # contiguous, so every chunk DMA is clean per-partition runs.
    adj_d = nc.dram_tensor("adj_p", [P, N_ROUNDS, NJT, W], fp8, kind="ExternalInput")
    # hi/lo stationaries interleaved by j-tile: [p][jtile][2][d] so a
    # leading slice of j-tiles is contiguous (head/tail split load).
    xs_d = nc.dram_tensor("xs_p", [P, NJT, 2, D], fp8, kind="ExternalInput")
    u_d = nc.dram_tensor("U", [D, D], f32r, kind="ExternalInput")
    # -U[0, :] as a per-partition bias column: the deg row rides through
    # the combine as "+1", the U-matmul turns it into +U[0,e], and the
    # ReLU's bias subtracts it back out.
    u0n_d = nc.dram_tensor("u0n", [D, 1], f32, kind="ExternalInput")
    # output [e, i_core] bf16; host transposes/upcasts.
    out_d = nc.dram_tensor("out_t", [P, I_CORE], bf16, kind="ExternalOutput")

    with tile.TileContext(nc, trace_sim=False) as tc, ExitStack() as ctx:
        const_pool = ctx.enter_context(tc.tile_pool(name="const", bufs=1))
        adj_pool = ctx.enter_context(tc.tile_pool(name="adj", bufs=8))
        scale_pool = ctx.enter_context(tc.tile_pool(name="scale", bufs=2))
        out_pool = ctx.enter_context(tc.tile_pool(name="out", bufs=2))
        small_pool = ctx.enter_context(tc.tile_pool(name="small", bufs=2))
        ps_a = ctx.enter_context(tc.tile_pool(name="ps_a", bufs=3, space="PSUM"))
        ps_b = ctx.enter_context(tc.tile_pool(name="ps_b", bufs=3, space="PSUM"))
        ps_o = ctx.enter_context(tc.tile_pool(name="ps_o", bufs=1, space="PSUM"))
        ps_w = ctx.enter_context(tc.tile_pool(name="ps_w", bufs=1, space="PSUM"))

        # --- warmup: ~3us of dummy matmuls flips the PE HAM clock-gate
        # to 2.4 GHz right as the first real chunk's semaphore fires, so
        # the real stream runs warm from its first matmul. ---
        warm_sb = const_pool.tile([P, 2, W], fp8)
        nc.vector.memset(warm_sb[:], 0.0)
        warm_ps = ps_w.tile([P, W], f32, tag="warm")
        for _ in range(7):
            nc.tensor.matmul(
                warm_ps[:],
                warm_sb[:, :, 0:D],
                warm_sb[:],
                start=True,
                stop=True,
                perf_mode=DR,
            )

        # --- constant loads: x head leads the adj (sync) queue so the
        # first matmuls are gated only by it + chunk 0; the x tail and U
        # ride the scalar queue in parallel. ---
        xs_sb = const_pool.tile([P, NJT, 2, D], fp8)
        nc.sync.dma_start(xs_sb[:, 0 : 2 * XS_HEAD], xs_d[:, 0 : 2 * XS_HEAD])
        u_sb = const_pool.tile([P, D], f32r)
        nc.scalar.dma_start(u_sb[:], u_d[:])
        u0n_sb = const_pool.tile([P, 1], f32)
        nc.scalar.dma_start(u0n_sb[:], u0n_d[:])
        nc.scalar.dma_start(
            xs_sb[:, 2 * XS_HEAD : 16], xs_d[:, 2 * XS_HEAD : 16]
        )
        nc.scalar.dma_start(xs_sb[:, 16:], xs_d[:, 16:])

        def emit_tail(q, a_ps, b_ps, split):
            """Round tail: combine hi+lo, 1/deg scale, U-matmul, ReLU, store.
            split=2 pipelines the chain in two column halves."""
            ws = W // split
            o_ps = ps_o.tile([P, W], f32, tag="ops")
            for h in range(split):
                cs = slice(h * ws, (h + 1) * ws)
                # A lands one matmul before B; its PSUM->SBUF copy can
                # start while B's last matmul drains.
                acp = scale_pool.tile([P, ws], f32, tag=f"acp{split}{h}")
                nc.vector.tensor_copy(acp[:], a_ps[:, cs])
                recip = small_pool.tile([1, ws], f32, tag=f"recip{split}{h}")
                nc.vector.reciprocal_approx_fast(recip[:], b_ps[0:1, cs])
                rb = scale_pool.tile([P, ws], f32, tag=f"rb{split}{h}")
                nc.gpsimd.partition_broadcast(rb[:], recip[:])
                # sum = A + B; row 0 becomes A[0] + deg, handled by the
                # ReLU bias below.
                sum_sb = scale_pool.tile([P, ws], f32, tag=f"sum{split}{h}")
                nc.vector.scalar_tensor_tensor(
                    sum_sb[:], b_ps[:, cs], 1.0, acp[:], MUL, ADD
                )
                aggs = scale_pool.tile([P, ws], f32r, tag=f"aggs{split}{h}")
                nc.vector.tensor_mul(aggs[:], sum_sb[:], rb[:])
                nc.tensor.matmul(o_ps[:, cs], u_sb[:], aggs[:], start=True, stop=True)
                out_sb = out_pool.tile([P, ws], bf16, tag=f"osb{split}{h}")
                nc.scalar.activation(
                    out_sb[:], o_ps[:, cs], RELU, bias=u0n_sb[:]
                )
                nc.scalar.dma_start(out_d[:, q * W + h * ws : q * W + (h + 1) * ws], out_sb[:])

        CHUNKS = [
            [1, 1, 2, 4, 8],
            [8, 8],
            [8, 8],
            [8, 4, 2, 2],
        ]
        pending = None
        for q in range(N_ROUNDS):
            a_ps = ps_a.tile([P, W], f32, tag="a")
            b_ps = ps_b.tile([P, W], f32, tag="b")
            chunk_pairs = CHUNKS[q]
            pt0 = 0
            for c, cp in enumerate(chunk_pairs):
                adj_sb = adj_pool.tile([P, 2 * cp, W], fp8, tag="adj")
                nc.sync.dma_start(
                    adj_sb[:], adj_d[:, q, 2 * pt0 : 2 * (pt0 + cp), :]
                )
                first, last = c == 0, c == len(chunk_pairs) - 1
                for u in range(cp):
                    pt = pt0 + u
                    mv = adj_sb[:, 2 * u : 2 * u + 2, :]
                    nc.tensor.matmul(
                        a_ps[:],
                        xs_sb[:, 2 * pt : 2 * pt + 2, 0, :],
                        mv,
                        start=(first and u == 0),
                        stop=(last and u == cp - 1),
                        perf_mode=DR,
                    )
                    nc.tensor.matmul(
                        b_ps[:],
                        xs_sb[:, 2 * pt : 2 * pt + 2, 1, :],
                        mv,
                        start=(first and u == 0),
                        stop=(last and u == cp - 1),
                        perf_mode=DR,
                    )
                pt0 += cp
                if pending is not None and c == 1:
                    emit_tail(*pending, split=1)
                    pending = None
            pending = (q, a_ps, b_ps)
        emit_tail(*pending, split=2)

    nc.compile()
    return nc


def _get_program():
    global _PROG
    if _PROG is None:
        _PROG = _build_program()
    return _PROG


E4 = ml_dtypes.float8_e4m3


def _shard_inputs(x, adj_mat, U):
    # adj -> fp8 via bit trick: 0/1 exact (1.0 == 0x38 in e4m3).
    adj8 = (adj_mat != 0).astype(np.uint8) * np.uint8(0x38)
    x32 = np.asarray(x, dtype=np.float32)
    in_maps = []
    for c in range(N_CORES):
        b, half = c // 2, c % 2
        i0 = half * I_CORE
        a = adj8[b, :, i0 : i0 + I_CORE]  # [N, I_CORE] uint8
        # [t*128+p, q*512+i] -> [p, q, t, i]
        a = np.ascontiguousarray(
            a.reshape(NJT, P, N_ROUNDS, W).transpose(1, 2, 0, 3)
        ).view(E4)
        xb = x32[b]  # [N, D]
        xh = xb.astype(E4)
        xl = (xb - xh.astype(np.float32)).astype(E4)
        xl[:, 0] = E4(1.0)  # ones column -> deg in PSUM B partition 0
        xs = np.stack([xh, xl], axis=1)  # [N, 2, D]
        # [t*128+p, s, d] -> [p, t, s, d]
        xs = np.ascontiguousarray(
            xs.reshape(NJT, P, 2, D).transpose(1, 0, 2, 3)
        )
        u32 = np.ascontiguousarray(U.astype(np.float32))
        u0n = np.ascontiguousarray(-u32[0, :].reshape(D, 1))
        in_maps.append({"adj_p": a, "xs_p": xs, "U": u32, "u0n": u0n})
    return in_maps


def _run(x, adj_mat, U, trace=False):
    from concourse.bass_utils import run_bass_kernel_spmd

    nc = _get_program()
    in_maps = _shard_inputs(x, adj_mat, U)
    res = run_bass_kernel_spmd(
        nc, in_maps, core_ids=list(range(N_CORES)), trace=trace
    )
    out = np.empty((B, N, D), dtype=np.float32)
    for c in range(N_CORES):
        b, half = c // 2, c % 2
        i0 = half * I_CORE
        ot = res.results[c]["out_t"]  # [128 e, I_CORE] bf16
        out[b, i0 : i0 + I_CORE, :] = ot.astype(np.float32).T
    return out, res


def kernel(x, adj_mat, U):
    out, _ = _run(
        np.asarray(x, dtype=np.float32),
        np.asarray(adj_mat, dtype=np.float32),
        np.asarray(U, dtype=np.float32),
    )
    return out
